# revision 2
# baseline (speedup 1.0000x reference)
"""Trainium2 Bass kernel for nn_CriticNetwork (GCN message passing + critic MLP).

Strategy (8 NeuronCores, SPMD, no collectives):
  - Only agg[agent_idx] rows are consumed downstream, so message passing is
    pruned to edges whose destination is an agent node (dead-code elimination).
  - GCN transform is algebraically moved after aggregation:
    A_hat @ (x W) == (A_hat @ x) W, so we aggregate 128-dim x rows.
  - Agents are sharded evenly: core c owns agents [c*2048, (c+1)*2048), sorted
    by indegree so fixed-K gather tiles are tight. Each core gathers the
    (dinv[src]-prescaled, bf16) x rows of its agents' in-edges with dma_gather
    (chunked across the 4 SWDGE queues to avoid descriptor-ring stalls),
    accumulates them with identity-matmuls into PSUM (giving agg^T
    feature-major), applies dinv[dst], then runs the critic head entirely
    feature-major with bf16 matmuls. LayerNorm mean-centering is folded into
    W1/W2 host-side (exact); the +b2c term is injected into the L2 PSUM via a
    rank-1 matmul with the std1 row so both LN variances reduce to a single
    ones-matrix matmul over squared activations.
  - Host does only index bookkeeping + weight folding; all O(E*dim) and
    O(A*dim^2) math runs on device.
"""
import os
import sys

sys.path.insert(0, "/opt/trn_rl_repo")

import numpy as np
import ml_dtypes

import concourse.bass as bass
import concourse.tile as tile
import concourse.mybir as mybir
from concourse import bacc
from concourse.bass_utils import run_bass_kernel_spmd

# ---- problem constants (hardcoded per spec) ----
N_NODES = 50000
DIM = 128          # IN_DIM
HID = 256
F1 = 1024
F2 = 512
NACT = 64
N_EDGES = 800000
N_AGENTS = 16384
N_CORES = 8
PA = N_AGENTS // N_CORES      # 2048 agents per core
TILES = PA // 128             # 16 d-tiles per core
GROUPS = 4                    # head processed in 4 groups of 512 agents
DG = PA // GROUPS             # 512
EPS = 1e-5
CHUNK_K = 12                  # max k-blocks (128 rows each) per dma_gather

F32 = mybir.dt.float32
F32R = mybir.dt.float32r
BF16 = mybir.dt.bfloat16
I16 = mybir.dt.int16
AF = mybir.ActivationFunctionType
OP = mybir.AluOpType

_KERNEL_CACHE = {}


def _wrap_idxs(idx_flat):
    """dma_gather index layout: index j lives at [j%16, j//16], replicated
    across the 8 groups of 16 partitions. idx_flat length must be %16==0."""
    arr = idx_flat.reshape(-1, 16).T.astype(np.int16)   # [16, n/16]
    return np.tile(arr, (8, 1))                          # [128, n/16]


def _preprocess(x, action, W_gcn, b_gcn, W1, b1, g1, beta1, W2, b2, g2, beta2,
                Wa, ba, Wq, bq, edge_index, agent_idx):
    f32 = np.float32
    x = np.asarray(x, f32); action = np.asarray(action, f32)
    edge_index = np.asarray(edge_index); agent_idx = np.asarray(agent_idx)
    W_gcn = np.asarray(W_gcn, f32); b_gcn = np.asarray(b_gcn, f32)
    W1 = np.asarray(W1, f32); b1 = np.asarray(b1, f32)
    g1 = np.asarray(g1, f32); beta1 = np.asarray(beta1, f32)
    W2 = np.asarray(W2, f32); b2 = np.asarray(b2, f32)
    g2 = np.asarray(g2, f32); beta2 = np.asarray(beta2, f32)
    Wa = np.asarray(Wa, f32); ba = np.asarray(ba, f32)
    Wq = np.asarray(Wq, f32); bq = np.asarray(bq, f32)

    assert np.all(beta1 == 0.0) and np.all(g1 > 0.0), \
        "kernel fast path requires beta1==0 and g1>0 (module init guarantees this)"

    N = N_NODES
    loops = np.arange(N, dtype=edge_index.dtype)
    src_all = np.concatenate([edge_index[0], loops])
    dst_all = np.concatenate([edge_index[1], loops])
    deg = np.bincount(dst_all, minlength=N).astype(np.int64)
    dinv = (1.0 / np.sqrt(np.maximum(deg, 1.0))).astype(f32)

    order = np.argsort(dst_all, kind="stable")
    src_sorted = src_all[order]
    starts = np.searchsorted(dst_all[order], np.arange(N + 1))

    # per-core agent partition + indegree sort
    perms, agents_p, indegs = [], [], []
    for c in range(N_CORES):
        ag = agent_idx[c * PA:(c + 1) * PA]
        ind = deg[ag]
        perm = np.argsort(ind, kind="stable")
        perms.append(perm)
        agents_p.append(ag[perm])
        indegs.append(ind[perm])

    # shared per-tile K (max over cores so the SPMD program is identical)
    K = np.zeros(TILES, np.int64)
    for c in range(N_CORES):
        K = np.maximum(K, indegs[c].reshape(TILES, 128).max(axis=1))
    K = np.maximum(K, 1).astype(int)
    koff = np.concatenate([[0], np.cumsum(K)])
    tot_k = int(koff[-1])

    # per-core edge tables (src node per slot; -1 = pad)
    slot_src = []   # [TILES] of [K[t], 128] global node ids (-1 pad)
    for c in range(N_CORES):
        ag = agents_p[c]; ind = indegs[c]
        per_tile = []
        for t in range(TILES):
            kt = K[t]
            tbl = np.full((kt, 128), -1, np.int64)
            for p in range(128):
                a = int(ag[t * 128 + p]); d = int(ind[t * 128 + p])
                s = starts[a]
                tbl[:d, p] = src_sorted[s:s + d]
            per_tile.append(tbl)
        slot_src.append(per_tile)

    # per-core compaction of source rows
    uniqs, n_us = [], []
    for c in range(N_CORES):
        allsrc = np.concatenate([t.ravel() for t in slot_src[c]])
        uniq = np.unique(allsrc[allsrc >= 0])
        uniqs.append(uniq); n_us.append(len(uniq))
    NSRC_PAD = max(n_us) + 1
    assert NSRC_PAD <= 32767, f"NSRC_PAD={NSRC_PAD} exceeds int16 index range"

    xsrc_list, idx_list, dinvd_list, actT_list = [], [], [], []
    for c in range(N_CORES):
        uniq = uniqs[c]; n_u = n_us[c]
        xs = np.zeros((NSRC_PAD, DIM), f32)
        xs[:n_u] = x[uniq] * dinv[uniq][:, None]
        xsrc_list.append(xs.astype(ml_dtypes.bfloat16))
        zero_idx = n_u
        # remap global src -> local compact index
        wrapped = []
        for t in range(TILES):
            tbl = slot_src[c][t]
            loc = np.searchsorted(uniq, np.maximum(tbl, 0))
            loc = np.where(tbl >= 0, loc, zero_idx).astype(np.int64)
            wrapped.append(_wrap_idxs(loc.ravel()))   # [128, K[t]*8]
        idx_list.append(np.concatenate(wrapped, axis=1).astype(np.int16))
        dinvd_list.append(np.broadcast_to(
            dinv[agents_p[c]].reshape(1, PA), (128, PA)).astype(f32))
        # augmented action^T: row 64 = ones (for the fused +ba+beta2 bias)
        at = np.ones((NACT + 1, PA), f32)
        at[:NACT] = action[c * PA:(c + 1) * PA][perms[c]].T
        actT_list.append(at.astype(ml_dtypes.bfloat16))

    # ---- weight folding (exact algebra) ----
    w1m = W1.mean(axis=1)                       # [HID]
    W1f = W1 - w1m[:, None]                     # zero col-mean
    b1c = b1 - b1.mean()
    W2g = g1[:, None] * W2
    w2gm = W2g.mean(axis=1)
    W2f = W2g - w2gm[:, None]
    b2c = b2 - b2.mean()
    bb = ba + beta2

    def ktile_pack(W, kt, fdim):   # [kt*128, fdim] -> [128, kt*fdim]
        return np.ascontiguousarray(
            W.reshape(kt, 128, fdim).transpose(1, 0, 2).reshape(128, kt * fdim))

    bf16 = ml_dtypes.bfloat16
    wa_aug = np.empty((NACT + 1, F2), f32)
    wa_aug[:NACT] = Wa
    wa_aug[NACT] = bb
    wqc = np.ascontiguousarray(Wq.reshape(4, 128).T)          # [128, 4]

    weights = {
        "wgcn": W_gcn.astype(bf16),                           # [128, 256]
        "w1": ktile_pack(W1f, 2, F1).astype(bf16),            # [128, 2048]
        "w2": ktile_pack(W2f, 8, F2).astype(bf16),            # [128, 4096]
        "wa": wa_aug.astype(bf16),                            # [65, 512]
        "wqc": wqc.astype(bf16),                              # [128, 4]
        "bgcn_col": np.ascontiguousarray(b_gcn.reshape(2, 128).T),
        "b1c32_col": np.ascontiguousarray((b1c / 32.0).reshape(8, 128).T),
        "b1_col": np.ascontiguousarray(b1c.reshape(8, 128).T),
        "g2_col": np.ascontiguousarray(g2.reshape(4, 128).T),
        "b2c_row": b2c.reshape(1, F2).astype(bf16),           # [1, 512]
        "onesmat_in": np.ones((128, 128), bf16),
        "ident_in": np.eye(128, dtype=bf16),
    }
    meta = dict(NSRC_PAD=NSRC_PAD, K=tuple(int(k) for k in K),
                koff=tuple(int(o) for o in koff), tot_k=tot_k,
                bq=float(bq[0]))
    percore = dict(xsrc=xsrc_list, idx=idx_list, dinvd=dinvd_list,
                   actT=actT_list)
    return weights, percore, perms, meta


def _build(meta):
    NSRC_PAD = meta["NSRC_PAD"]; K = meta["K"]; koff = meta["koff"]
    tot_k = meta["tot_k"]; bq = meta["bq"]
    KMAX = max(K)

    nc = bacc.Bacc("TRN2", target_bir_lowering=False, debug=False,
                   num_devices=N_CORES, num_swdge_queues=4,
                   dynamic_dma_scratch_size=32768)
    dram = {}
    def din(name, shape, dt):
        dram[name] = nc.dram_tensor(name, shape, dt, kind="ExternalInput").ap()
        return dram[name]

    xsrc = din("xsrc", [NSRC_PAD, DIM], BF16)
    idxs = din("idx", [128, 8 * tot_k], I16)
    dinvd = din("dinvd", [128, PA], F32)
    actT_d = din("actT", [NACT + 1, PA], BF16)
    wgcn_d = din("wgcn", [128, HID], BF16)
    w1_d = din("w1", [128, 2 * F1], BF16)
    w2_d = din("w2", [128, 8 * F2], BF16)
    wa_d = din("wa", [NACT + 1, F2], BF16)
    wqc_d = din("wqc", [128, 4], BF16)
    bgcn_d = din("bgcn_col", [128, 2], F32)
    b1c32_d = din("b1c32_col", [128, 8], F32)
    b1_d = din("b1_col", [128, 8], F32)
    g2_d = din("g2_col", [128, 4], F32)
    b2c_d = din("b2c_row", [1, F2], BF16)
    onesmat_d = din("onesmat_in", [128, 128], BF16)
    ident_d = din("ident_in", [128, 128], BF16)
    OUT = nc.dram_tensor("q", [PA, 1], F32, kind="ExternalOutput").ap()

    with tile.TileContext(nc) as tc:
        with tc.tile_pool(name="w", bufs=1) as wp, \
             tc.tile_pool(name="edges", bufs=5) as ep, \
             tc.tile_pool(name="zp", bufs=3) as zp, \
             tc.tile_pool(name="s1p", bufs=10) as s1p, \
             tc.tile_pool(name="sqp", bufs=2) as sqp, \
             tc.tile_pool(name="yap", bufs=5) as yap, \
             tc.tile_pool(name="ya2p", bufs=2) as ya2p, \
             tc.tile_pool(name="tlp", bufs=4) as tlp, \
             tc.tile_pool(name="sap", bufs=5) as sap, \
             tc.tile_pool(name="vec", bufs=4) as vec, \
             tc.tile_pool(name="bcp", bufs=4) as bcp, \
             tc.tile_pool(name="ps", bufs=1, space="PSUM") as pp:

            # ---------- preload (HWDGE on sync+scalar; gpsimd kept free) ----
            idxt = wp.tile([128, 8 * tot_k], I16); nc.sync.dma_start(idxt[:], idxs[:])
            w2 = wp.tile([128, 8 * F2], BF16); nc.sync.dma_start(w2[:], w2_d[:])
            wa = wp.tile([NACT + 1, F2], BF16); nc.sync.dma_start(wa[:], wa_d[:])
            wqc = wp.tile([128, 4], BF16); nc.sync.dma_start(wqc[:], wqc_d[:])
            bgcn = wp.tile([128, 2], F32); nc.sync.dma_start(bgcn[:], bgcn_d[:])
            b1c32 = wp.tile([128, 8], F32); nc.sync.dma_start(b1c32[:], b1c32_d[:])
            b1c = wp.tile([128, 8], F32); nc.sync.dma_start(b1c[:], b1_d[:])
            g2c = wp.tile([128, 4], F32); nc.sync.dma_start(g2c[:], g2_d[:])
            b2cr = wp.tile([1, F2], BF16); nc.sync.dma_start(b2cr[:], b2c_d[:])

            w1 = wp.tile([128, 2 * F1], BF16); nc.scalar.dma_start(w1[:], w1_d[:])
            wgcn = wp.tile([128, HID], BF16); nc.scalar.dma_start(wgcn[:], wgcn_d[:])
            actT = wp.tile([NACT + 1, PA], BF16); nc.scalar.dma_start(actT[:], actT_d[:])
            dinvd_b = wp.tile([128, PA], F32); nc.scalar.dma_start(dinvd_b[:], dinvd[:])
            onesm = wp.tile([128, 128], BF16); nc.scalar.dma_start(onesm[:], onesmat_d[:])
            ident = wp.tile([128, 128], BF16); nc.scalar.dma_start(ident[:], ident_d[:])

            eps_t = wp.tile([128, 1], F32); nc.gpsimd.memset(eps_t[:], EPS)
            aggb = wp.tile([128, PA], BF16)      # agg^T, feature-major
            qout = wp.tile([1, PA], F32)

            qn = [0]   # SWDGE queue round-robin counter

            for g in range(GROUPS):
                gs0 = g * DG
                # ---------- aggregation for this group's 4 d-tiles ----------
                for tl in range(4):
                    t = g * 4 + tl
                    kt = K[t]
                    e = ep.tile([128, KMAX * 128], BF16, tag="edges")
                    nchunk = (kt + CHUNK_K - 1) // CHUNK_K
                    bounds = [kt * i // nchunk for i in range(nchunk + 1)]
                    for ci in range(nchunk):
                        c0, c1 = bounds[ci], bounds[ci + 1]
                        e3 = e[:, c0 * 128:c1 * 128].rearrange(
                            "p (k e) -> p k e", e=128)
                        nc.gpsimd.dma_gather(
                            e3, xsrc[:],
                            idxt[:, 8 * (koff[t] + c0):8 * (koff[t] + c1)],
                            128 * (c1 - c0), 128 * (c1 - c0), DIM,
                            single_packet=False, queue_num=qn[0] % 4)
                        qn[0] += 1
                    aps = pp.tile([128, 128], F32, tag="agg", bufs=2)
                    for k in range(kt):
                        nc.tensor.matmul(aps[:], e[:, k * 128:(k + 1) * 128],
                                         ident[:], start=(k == 0),
                                         stop=(k == kt - 1))
                    nc.vector.tensor_tensor(
                        aggb[:, t * 128:(t + 1) * 128], aps[:],
                        dinvd_b[:, t * 128:(t + 1) * 128], OP.mult)

                # ---------- transform: z = relu(W_gcn^T aggT + b_gcn) ----------
                zt = []
                for h in range(2):
                    zps = pp.tile([128, DG], F32, tag="big", bufs=3)
                    nc.tensor.matmul(zps[:], wgcn[:, h * 128:(h + 1) * 128],
                                     aggb[:, gs0:gs0 + DG], start=True, stop=True)
                    z = zp.tile([128, DG], BF16, tag="z")
                    nc.scalar.activation(z[:], zps[:], AF.Relu,
                                         bias=bgcn[:, h:h + 1])
                    zt.append(z)

                # ---------- L1 + LN1 (mean folded into W1f/b1c) ----------
                # var1 = sum over F1 of ((x1c + b1c)/32)^2  (1/32^2 = 1/F1)
                ps_sq1 = pp.tile([128, DG], F32, tag="stat", bufs=2)
                s1r = []
                for c in range(8):
                    lp = pp.tile([128, DG], F32, tag="big", bufs=3)
                    nc.tensor.matmul(lp[:], w1[:, c * 128:c * 128 + 128],
                                     zt[0][:], start=True, stop=False)
                    nc.tensor.matmul(lp[:], w1[:, F1 + c * 128:F1 + c * 128 + 128],
                                     zt[1][:], start=False, stop=True)
                    sq = sqp.tile([128, DG], BF16, tag="sq")
                    nc.scalar.activation(sq[:], lp[:], AF.Square,
                                         bias=b1c32[:, c:c + 1], scale=1.0 / 32.0)
                    nc.tensor.matmul(ps_sq1[:], onesm[:], sq[:],
                                     start=(c == 0), stop=(c == 7))
                    sr = s1p.tile([128, DG], BF16, tag="s1")
                    nc.scalar.activation(sr[:], lp[:], AF.Relu,
                                         bias=b1c[:, c:c + 1])
                    s1r.append(sr)

                std1 = vec.tile([128, DG], F32, tag="v")
                nc.scalar.activation(std1[:], ps_sq1[:], AF.Sqrt, bias=eps_t[:])
                rstd1b = bcp.tile([128, DG], F32, tag="bc")
                nc.vector.reciprocal_approx_fast(rstd1b[:], std1[:])
                std1row = vec.tile([1, DG], BF16, tag="vrow")
                nc.scalar.activation(std1row[:], std1[0:1, :], AF.Copy)

                # ---------- L2 (+rank-1 b2c*std1) + LN2 stats ----------
                # yb = (W2f@s1r + b2c x std1) * rstd1  ==  x2c + b2c  (zero-mean)
                # var2 = sum over F2 of (yb/sqrt(F2))^2
                ps_s2 = pp.tile([128, DG], F32, tag="stat", bufs=2)
                Yb = []
                for c2 in range(4):
                    lp = pp.tile([128, DG], F32, tag="big", bufs=3)
                    for kt8 in range(8):
                        nc.tensor.matmul(
                            lp[:], w2[:, kt8 * F2 + c2 * 128:kt8 * F2 + c2 * 128 + 128],
                            s1r[kt8][:], start=(kt8 == 0), stop=False)
                    nc.tensor.matmul(lp[:], b2cr[:, c2 * 128:(c2 + 1) * 128],
                                     std1row[:], start=False, stop=True)
                    yb = yap.tile([128, DG], BF16, tag="ya")
                    nc.vector.tensor_tensor(yb[:], lp[:], rstd1b[:], OP.mult)
                    y2 = ya2p.tile([128, DG], BF16, tag="ya2")
                    nc.scalar.activation(y2[:], yb[:], AF.Square,
                                         scale=float(F2) ** -0.5)
                    nc.tensor.matmul(ps_s2[:], onesm[:], y2[:],
                                     start=(c2 == 0), stop=(c2 == 3))
                    Yb.append(yb)

                std2 = vec.tile([128, DG], F32, tag="v")
                nc.scalar.activation(std2[:], ps_s2[:], AF.Sqrt, bias=eps_t[:])
                rstd2b = bcp.tile([128, DG], F32, tag="bc")
                nc.vector.reciprocal_approx_fast(rstd2b[:], std2[:])

                # ---------- tail: sa = relu(g2*yb*rstd2 + (pa+bb)) ----------
                qrow = pp.tile([1, DG], F32, tag="q", bufs=1)
                for c2 in range(4):
                    pa = pp.tile([128, DG], F32, tag="big", bufs=3)
                    nc.tensor.matmul(pa[:], wa[:, c2 * 128:(c2 + 1) * 128],
                                     actT[:, gs0:gs0 + DG], start=True, stop=True)
                    t2 = tlp.tile([128, DG], F32, tag="tl")
                    nc.vector.tensor_tensor(t2[:], Yb[c2][:], rstd2b[:], OP.mult)
                    t3 = tlp.tile([128, DG], F32, tag="tl")
                    nc.vector.scalar_tensor_tensor(t3[:], t2[:], g2c[:, c2:c2 + 1],
                                                   pa[:], OP.mult, OP.add)
                    sa = sap.tile([128, DG], BF16, tag="sa")
                    nc.scalar.activation(sa[:], t3[:], AF.Relu)
                    nc.tensor.matmul(qrow[:], wqc[:, c2:c2 + 1], sa[:],
                                     start=(c2 == 0), stop=(c2 == 3))

                nc.scalar.activation(qout[0:1, gs0:gs0 + DG], qrow[:],
                                     AF.Copy, bias=bq)

            out_ap = OUT.rearrange("(a b) o -> b (a o)", b=1)
            nc.sync.dma_start(out_ap, qout[:])
    nc.compile()
    return nc


def kernel(**inputs):
    weights, percore, perms, meta = _preprocess(**inputs)

    key = (meta["NSRC_PAD"], meta["K"], meta["tot_k"])
    if key not in _KERNEL_CACHE:
        _KERNEL_CACHE[key] = _build(meta)
    nc = _KERNEL_CACHE[key]

    in_maps = []
    for c in range(N_CORES):
        m = dict(weights)
        m["xsrc"] = percore["xsrc"][c]
        m["idx"] = percore["idx"][c]
        m["dinvd"] = percore["dinvd"][c]
        m["actT"] = percore["actT"][c]
        in_maps.append(m)

    trace = os.environ.get("KERNEL_TRACE", "0") == "1"
    kw = {}
    if trace:
        import types, contextlib, ctypes
        if "antenv.axon_hooks" not in sys.modules:
            lib = ctypes.CDLL("/opt/axon/libaxon_pjrt.so")
            lib.axon_start_nrt_profile.argtypes = [
                ctypes.POINTER(ctypes.c_int64), ctypes.c_size_t]
            lib.axon_start_nrt_profile.restype = ctypes.c_int64
            lib.axon_stop_nrt_profile.argtypes = [ctypes.c_char_p]
            lib.axon_stop_nrt_profile.restype = ctypes.c_int64

            @contextlib.contextmanager
            def _hook(output_dir, device_ids):
                import jax
                jax.devices()
                if device_ids:
                    ids = (ctypes.c_int64 * len(device_ids))(*device_ids)
                    rc = lib.axon_start_nrt_profile(ids, len(device_ids))
                else:
                    rc = lib.axon_start_nrt_profile(None, 0)
                if rc != 0:
                    raise RuntimeError(f"axon_start_nrt_profile rc={rc}")
                try:
                    yield
                finally:
                    n = lib.axon_stop_nrt_profile(str(output_dir).encode())
                    print(f"profile: {n} file(s) written to {output_dir}",
                          file=sys.stderr)

            mod = types.ModuleType("antenv.axon_hooks")
            mod.get_axon_ntff_profile_hook = lambda: _hook
            sys.modules["antenv.axon_hooks"] = mod
        kw = dict(trace=True,
                  tmpdir=os.environ.get("KERNEL_TRACE_DIR") or None)

    res = run_bass_kernel_spmd(nc, in_maps, list(range(N_CORES)), **kw)
    if trace and res.exec_time_ns is not None:
        print(f"HW exec time: {res.exec_time_ns} ns")

    out = np.empty((N_AGENTS, 1), np.float32)
    for c in range(N_CORES):
        q = res.results[c]["q"]          # [PA, 1], indegree-sorted order
        blk = out[c * PA:(c + 1) * PA]
        blk[perms[c]] = q
    return out


# revision 7
# speedup vs baseline: 1.0068x; 1.0068x over previous
"""Trainium2 Bass kernel for nn_CriticNetwork (GCN message passing + critic MLP).

Strategy (8 NeuronCores, SPMD, no collectives):
  - Only agg[agent_idx] rows are consumed downstream, so message passing is
    pruned to edges whose destination is an agent node (dead-code elimination).
  - GCN transform is algebraically moved after aggregation:
    A_hat @ (x W) == (A_hat @ x) W, so we aggregate 128-dim x rows.
  - Agents are sharded evenly: core c owns agents [c*2048, (c+1)*2048), sorted
    by indegree so fixed-K gather tiles are tight. Each core gathers the
    (dinv[src]-prescaled, bf16) x rows of its agents' in-edges with dma_gather
    (chunked across the 4 SWDGE queues to avoid descriptor-ring stalls),
    accumulates them with identity-matmuls into PSUM (giving agg^T
    feature-major), applies dinv[dst], then runs the critic head entirely
    feature-major with bf16 matmuls. LayerNorm mean-centering is folded into
    W1/W2 host-side (exact); the +b2c term is injected into the L2 PSUM via a
    rank-1 matmul with the std1 row so both LN variances reduce to a single
    ones-matrix matmul over squared activations.
  - Host does only index bookkeeping + weight folding; all O(E*dim) and
    O(A*dim^2) math runs on device.
"""
import os
import sys

sys.path.insert(0, "/opt/trn_rl_repo")

import numpy as np
import ml_dtypes

import concourse.bass as bass
import concourse.tile as tile
import concourse.mybir as mybir
from concourse import bacc
from concourse.bass_utils import run_bass_kernel_spmd

# ---- problem constants (hardcoded per spec) ----
N_NODES = 50000
DIM = 128          # IN_DIM
HID = 256
F1 = 1024
F2 = 512
NACT = 64
N_EDGES = 800000
N_AGENTS = 16384
N_CORES = 8
PA = N_AGENTS // N_CORES      # 2048 agents per core
TILES = PA // 128             # 16 d-tiles per core
GROUPS = 4                    # head processed in 4 groups of 512 agents
DG = PA // GROUPS             # 512
EPS = 1e-5
CHUNK_K = 12                  # max k-blocks (128 rows each) per dma_gather

F32 = mybir.dt.float32
F32R = mybir.dt.float32r
BF16 = mybir.dt.bfloat16
I16 = mybir.dt.int16
AF = mybir.ActivationFunctionType
OP = mybir.AluOpType

_KERNEL_CACHE = {}


def _wrap_idxs(idx_flat):
    """dma_gather index layout: index j lives at [j%16, j//16], replicated
    across the 8 groups of 16 partitions. idx_flat length must be %16==0."""
    arr = idx_flat.reshape(-1, 16).T.astype(np.int16)   # [16, n/16]
    return np.tile(arr, (8, 1))                          # [128, n/16]


def _preprocess(x, action, W_gcn, b_gcn, W1, b1, g1, beta1, W2, b2, g2, beta2,
                Wa, ba, Wq, bq, edge_index, agent_idx):
    f32 = np.float32
    x = np.asarray(x, f32); action = np.asarray(action, f32)
    edge_index = np.asarray(edge_index); agent_idx = np.asarray(agent_idx)
    W_gcn = np.asarray(W_gcn, f32); b_gcn = np.asarray(b_gcn, f32)
    W1 = np.asarray(W1, f32); b1 = np.asarray(b1, f32)
    g1 = np.asarray(g1, f32); beta1 = np.asarray(beta1, f32)
    W2 = np.asarray(W2, f32); b2 = np.asarray(b2, f32)
    g2 = np.asarray(g2, f32); beta2 = np.asarray(beta2, f32)
    Wa = np.asarray(Wa, f32); ba = np.asarray(ba, f32)
    Wq = np.asarray(Wq, f32); bq = np.asarray(bq, f32)

    assert np.all(beta1 == 0.0) and np.all(g1 > 0.0), \
        "kernel fast path requires beta1==0 and g1>0 (module init guarantees this)"

    N = N_NODES
    loops = np.arange(N, dtype=edge_index.dtype)
    src_all = np.concatenate([edge_index[0], loops])
    dst_all = np.concatenate([edge_index[1], loops])
    deg = np.bincount(dst_all, minlength=N).astype(np.int64)
    dinv = (1.0 / np.sqrt(np.maximum(deg, 1.0))).astype(f32)

    order = np.argsort(dst_all, kind="stable")
    src_sorted = src_all[order]
    starts = np.searchsorted(dst_all[order], np.arange(N + 1))

    # per-core agent partition + indegree sort
    perms, agents_p, indegs = [], [], []
    for c in range(N_CORES):
        ag = agent_idx[c * PA:(c + 1) * PA]
        ind = deg[ag]
        perm = np.argsort(ind, kind="stable")
        perms.append(perm)
        agents_p.append(ag[perm])
        indegs.append(ind[perm])

    # shared per-tile K (max over cores so the SPMD program is identical)
    K = np.zeros(TILES, np.int64)
    for c in range(N_CORES):
        K = np.maximum(K, indegs[c].reshape(TILES, 128).max(axis=1))
    K = np.maximum(K, 1).astype(int)
    koff = np.concatenate([[0], np.cumsum(K)])
    tot_k = int(koff[-1])

    # per-core edge tables (src node per slot; -1 = pad)
    slot_src = []   # [TILES] of [K[t], 128] global node ids (-1 pad)
    for c in range(N_CORES):
        ag = agents_p[c]; ind = indegs[c]
        per_tile = []
        for t in range(TILES):
            kt = K[t]
            tbl = np.full((kt, 128), -1, np.int64)
            for p in range(128):
                a = int(ag[t * 128 + p]); d = int(ind[t * 128 + p])
                s = starts[a]
                tbl[:d, p] = src_sorted[s:s + d]
            per_tile.append(tbl)
        slot_src.append(per_tile)

    # per-core compaction of source rows
    uniqs, n_us = [], []
    for c in range(N_CORES):
        allsrc = np.concatenate([t.ravel() for t in slot_src[c]])
        uniq = np.unique(allsrc[allsrc >= 0])
        uniqs.append(uniq); n_us.append(len(uniq))
    NSRC_PAD = max(n_us) + 1
    assert NSRC_PAD <= 32767, f"NSRC_PAD={NSRC_PAD} exceeds int16 index range"

    xsrc_list, idx_list, dinvd_list, actT_list = [], [], [], []
    for c in range(N_CORES):
        uniq = uniqs[c]; n_u = n_us[c]
        xs = np.zeros((NSRC_PAD, DIM), f32)
        xs[:n_u] = x[uniq] * dinv[uniq][:, None]
        xsrc_list.append(xs.astype(ml_dtypes.bfloat16))
        zero_idx = n_u
        # remap global src -> local compact index
        wrapped = []
        for t in range(TILES):
            tbl = slot_src[c][t]
            loc = np.searchsorted(uniq, np.maximum(tbl, 0))
            loc = np.where(tbl >= 0, loc, zero_idx).astype(np.int64)
            wrapped.append(_wrap_idxs(loc.ravel()))   # [128, K[t]*8]
        idx_list.append(np.concatenate(wrapped, axis=1).astype(np.int16))
        dinvd_list.append(np.broadcast_to(
            dinv[agents_p[c]].reshape(1, PA), (128, PA)).astype(f32))
        # augmented action^T: row 64 = ones (for the fused +ba+beta2 bias)
        at = np.ones((NACT + 1, PA), f32)
        at[:NACT] = action[c * PA:(c + 1) * PA][perms[c]].T
        actT_list.append(at.astype(ml_dtypes.bfloat16))

    # ---- weight folding (exact algebra) ----
    w1m = W1.mean(axis=1)                       # [HID]
    W1f = W1 - w1m[:, None]                     # zero col-mean
    b1c = b1 - b1.mean()
    W2g = g1[:, None] * W2
    w2gm = W2g.mean(axis=1)
    W2f = W2g - w2gm[:, None]
    b2c = b2 - b2.mean()
    bb = ba + beta2

    def ktile_pack(W, kt, fdim):   # [kt*128, fdim] -> [128, kt*fdim]
        return np.ascontiguousarray(
            W.reshape(kt, 128, fdim).transpose(1, 0, 2).reshape(128, kt * fdim))

    bf16 = ml_dtypes.bfloat16
    wa_aug = np.empty((NACT + 1, F2), f32)
    wa_aug[:NACT] = Wa
    wa_aug[NACT] = bb
    wqc = np.ascontiguousarray(Wq.reshape(4, 128).T)          # [128, 4]

    weights = {
        "wgcn": W_gcn.astype(bf16),                           # [128, 256]
        "w1": ktile_pack(W1f, 2, F1).astype(bf16),            # [128, 2048]
        "w2": ktile_pack(W2f, 8, F2).astype(bf16),            # [128, 4096]
        "wa": wa_aug.astype(bf16),                            # [65, 512]
        "wqc": wqc.astype(bf16),                              # [128, 4]
        "bgcn_col": np.ascontiguousarray(b_gcn.reshape(2, 128).T),
        "b1c32_col": np.ascontiguousarray((b1c / 32.0).reshape(8, 128).T),
        "b1_col": np.ascontiguousarray(b1c.reshape(8, 128).T),
        "g2_col": np.ascontiguousarray(g2.reshape(4, 128).T),
        "b2c_row": b2c.reshape(1, F2).astype(bf16),           # [1, 512]
        "onesmat_in": np.ones((128, 128), bf16),
        "onesf2_in": np.full((128, 128), 1.0 / F2, bf16),     # 1/512 exact
        "ident_in": np.eye(128, dtype=bf16),
    }
    meta = dict(NSRC_PAD=NSRC_PAD, K=tuple(int(k) for k in K),
                koff=tuple(int(o) for o in koff), tot_k=tot_k,
                bq=float(bq[0]))
    percore = dict(xsrc=xsrc_list, idx=idx_list, dinvd=dinvd_list,
                   actT=actT_list)
    return weights, percore, perms, meta


def _build(meta):
    NSRC_PAD = meta["NSRC_PAD"]; K = meta["K"]; koff = meta["koff"]
    tot_k = meta["tot_k"]; bq = meta["bq"]
    KMAX = max(K)

    nc = bacc.Bacc("TRN2", target_bir_lowering=False, debug=False,
                   num_devices=N_CORES, num_swdge_queues=4,
                   dynamic_dma_scratch_size=32768)
    dram = {}
    def din(name, shape, dt):
        dram[name] = nc.dram_tensor(name, shape, dt, kind="ExternalInput").ap()
        return dram[name]

    KA = koff[4]      # idx columns for tiles 0-3 (loaded first, tiny)
    xsrc = din("xsrc", [NSRC_PAD, DIM], BF16)
    idxs = din("idx", [128, 8 * tot_k], I16)
    dinvd = din("dinvd", [128, PA], F32)
    actT_d = din("actT", [NACT + 1, PA], BF16)
    wgcn_d = din("wgcn", [128, HID], BF16)
    w1_d = din("w1", [128, 2 * F1], BF16)
    w2_d = din("w2", [128, 8 * F2], BF16)
    wa_d = din("wa", [NACT + 1, F2], BF16)
    wqc_d = din("wqc", [128, 4], BF16)
    bgcn_d = din("bgcn_col", [128, 2], F32)
    b1c32_d = din("b1c32_col", [128, 8], F32)
    b1_d = din("b1_col", [128, 8], F32)
    g2_d = din("g2_col", [128, 4], F32)
    b2c_d = din("b2c_row", [1, F2], BF16)
    onesmat_d = din("onesmat_in", [128, 128], BF16)
    onesf2_d = din("onesf2_in", [128, 128], BF16)
    ident_d = din("ident_in", [128, 128], BF16)
    OUT = nc.dram_tensor("q", [PA, 1], F32, kind="ExternalOutput").ap()

    with tile.TileContext(nc) as tc:
        with tc.tile_pool(name="w", bufs=1) as wp, \
             tc.tile_pool(name="edges", bufs=5) as ep, \
             tc.tile_pool(name="zp", bufs=4) as zp, \
             tc.tile_pool(name="s1p", bufs=12) as s1p, \
             tc.tile_pool(name="sqp", bufs=2) as sqp, \
             tc.tile_pool(name="yap", bufs=5) as yap, \
             tc.tile_pool(name="ya2p", bufs=2) as ya2p, \
             tc.tile_pool(name="tlp", bufs=4) as tlp, \
             tc.tile_pool(name="sap", bufs=5) as sap, \
             tc.tile_pool(name="vec", bufs=4) as vec, \
             tc.tile_pool(name="bcp", bufs=4) as bcp, \
             tc.tile_pool(name="ps", bufs=1, space="PSUM") as pp:

            # ---------- preload (HWDGE on sync+scalar; gpsimd kept free) ----
            # idx table for tiles 0-3 first so gathers start immediately
            idxA = wp.tile([128, 8 * KA], I16)
            nc.sync.dma_start(idxA[:], idxs[:, :8 * KA])
            idxB = wp.tile([128, 8 * (tot_k - KA)], I16)
            nc.sync.dma_start(idxB[:], idxs[:, 8 * KA:])
            w2 = wp.tile([128, 8 * F2], BF16); nc.sync.dma_start(w2[:], w2_d[:])
            wa = wp.tile([NACT + 1, F2], BF16); nc.sync.dma_start(wa[:], wa_d[:])
            wqc = wp.tile([128, 4], BF16); nc.sync.dma_start(wqc[:], wqc_d[:])
            bgcn = wp.tile([128, 2], F32); nc.sync.dma_start(bgcn[:], bgcn_d[:])
            b1c32 = wp.tile([128, 8], F32); nc.sync.dma_start(b1c32[:], b1c32_d[:])
            b1c = wp.tile([128, 8], F32); nc.sync.dma_start(b1c[:], b1_d[:])
            g2c = wp.tile([128, 4], F32); nc.sync.dma_start(g2c[:], g2_d[:])
            b2cr = wp.tile([1, F2], BF16); nc.sync.dma_start(b2cr[:], b2c_d[:])

            ident = wp.tile([128, 128], BF16); nc.scalar.dma_start(ident[:], ident_d[:])
            w1 = wp.tile([128, 2 * F1], BF16); nc.scalar.dma_start(w1[:], w1_d[:])
            wgcn = wp.tile([128, HID], BF16); nc.scalar.dma_start(wgcn[:], wgcn_d[:])
            actT = wp.tile([NACT + 1, PA], BF16); nc.scalar.dma_start(actT[:], actT_d[:])
            dinvd_b = wp.tile([128, PA], F32); nc.scalar.dma_start(dinvd_b[:], dinvd[:])
            onesm = wp.tile([128, 128], BF16); nc.scalar.dma_start(onesm[:], onesmat_d[:])
            onesf2 = wp.tile([128, 128], BF16); nc.scalar.dma_start(onesf2[:], onesf2_d[:])

            eps_t = wp.tile([128, 1], F32); nc.gpsimd.memset(eps_t[:], EPS)
            aggb = wp.tile([128, PA], BF16)      # agg^T, feature-major
            qout = wp.tile([1, PA], F32)

            qn = [0]   # SWDGE queue round-robin counter

            def idx_ap(col0, col1):
                if col1 <= KA:
                    return idxA[:, 8 * col0:8 * col1]
                return idxB[:, 8 * (col0 - KA):8 * (col1 - KA)]

            def stage_A(g):
                """gathers + identity-matmul aggregation + z transform"""
                gs0 = g * DG
                for tl in range(4):
                    t = g * 4 + tl
                    kt = K[t]
                    e = ep.tile([128, KMAX * 128], BF16, tag="edges")
                    nchunk = (kt + CHUNK_K - 1) // CHUNK_K
                    bounds = [kt * i // nchunk for i in range(nchunk + 1)]
                    for ci in range(nchunk):
                        c0, c1 = bounds[ci], bounds[ci + 1]
                        e3 = e[:, c0 * 128:c1 * 128].rearrange(
                            "p (k e) -> p k e", e=128)
                        nc.gpsimd.dma_gather(
                            e3, xsrc[:], idx_ap(koff[t] + c0, koff[t] + c1),
                            128 * (c1 - c0), 128 * (c1 - c0), DIM,
                            single_packet=False, queue_num=qn[0] % 4)
                        qn[0] += 1
                    aps = pp.tile([128, 128], F32, tag="agg", bufs=2)
                    for k in range(kt):
                        nc.tensor.matmul(aps[:], e[:, k * 128:(k + 1) * 128],
                                         ident[:], start=(k == 0),
                                         stop=(k == kt - 1))
                    nc.vector.tensor_tensor(
                        aggb[:, t * 128:(t + 1) * 128], aps[:],
                        dinvd_b[:, t * 128:(t + 1) * 128], OP.mult)
                zt = []
                for h in range(2):
                    zps = pp.tile([128, DG], F32, tag="big", bufs=3)
                    nc.tensor.matmul(zps[:], wgcn[:, h * 128:(h + 1) * 128],
                                     aggb[:, gs0:gs0 + DG], start=True, stop=True)
                    z = zp.tile([128, DG], BF16, tag="z")
                    nc.scalar.activation(z[:], zps[:], AF.Relu,
                                         bias=bgcn[:, h:h + 1])
                    zt.append(z)
                return zt

            def stage_B(g, zt):
                """L1 + LN1 stats (mean folded into W1f/b1c)
                var1 = sum over F1 of ((x1c + b1c)/32)^2  (1/32^2 = 1/F1)"""
                ps_sq1 = pp.tile([128, DG], F32, tag="stat", bufs=2)
                s1r = []
                for c in range(8):
                    lp = pp.tile([128, DG], F32, tag="big", bufs=3)
                    nc.tensor.matmul(lp[:], w1[:, c * 128:c * 128 + 128],
                                     zt[0][:], start=True, stop=False)
                    nc.tensor.matmul(lp[:], w1[:, F1 + c * 128:F1 + c * 128 + 128],
                                     zt[1][:], start=False, stop=True)
                    sq = sqp.tile([128, DG], BF16, tag="sq")
                    nc.scalar.activation(sq[:], lp[:], AF.Square,
                                         bias=b1c32[:, c:c + 1], scale=1.0 / 32.0)
                    nc.tensor.matmul(ps_sq1[:], onesm[:], sq[:],
                                     start=(c == 0), stop=(c == 7))
                    sr = s1p.tile([128, DG], BF16, tag="s1")
                    nc.scalar.activation(sr[:], lp[:], AF.Relu,
                                         bias=b1c[:, c:c + 1])
                    s1r.append(sr)

                std1 = vec.tile([128, DG], F32, tag="v")
                nc.scalar.activation(std1[:], ps_sq1[:], AF.Sqrt, bias=eps_t[:])
                rstd1b = bcp.tile([128, DG], F32, tag="bc")
                nc.vector.reciprocal_approx_fast(rstd1b[:], std1[:])
                std1row = vec.tile([1, DG], BF16, tag="vrow", bufs=3)
                nc.scalar.activation(std1row[:], std1[0:1, :], AF.Copy)
                return s1r, rstd1b, std1row

            def stage_C(g, s1r, rstd1b, std1row):
                """L2 (+rank-1 b2c*std1) + LN2 + tail + q
                yb = (W2f@s1r + b2c x std1) * rstd1 == x2c + b2c (zero-mean)
                var2 = (ones/F2) @ yb^2"""
                gs0 = g * DG
                ps_s2 = pp.tile([128, DG], F32, tag="stat", bufs=2)
                Yb = []
                for c2 in range(4):
                    lp = pp.tile([128, DG], F32, tag="big", bufs=3)
                    for kt8 in range(8):
                        nc.tensor.matmul(
                            lp[:], w2[:, kt8 * F2 + c2 * 128:kt8 * F2 + c2 * 128 + 128],
                            s1r[kt8][:], start=(kt8 == 0), stop=False)
                    nc.tensor.matmul(lp[:], b2cr[:, c2 * 128:(c2 + 1) * 128],
                                     std1row[:], start=False, stop=True)
                    yb = yap.tile([128, DG], BF16, tag="ya")
                    nc.vector.tensor_tensor(yb[:], lp[:], rstd1b[:], OP.mult)
                    y2 = ya2p.tile([128, DG], BF16, tag="ya2")
                    nc.vector.tensor_tensor(y2[:], yb[:], yb[:], OP.mult)
                    nc.tensor.matmul(ps_s2[:], onesf2[:], y2[:],
                                     start=(c2 == 0), stop=(c2 == 3))
                    Yb.append(yb)

                std2 = vec.tile([128, DG], F32, tag="v")
                nc.scalar.activation(std2[:], ps_s2[:], AF.Sqrt, bias=eps_t[:])
                rstd2b = bcp.tile([128, DG], F32, tag="bc")
                nc.vector.reciprocal_approx_fast(rstd2b[:], std2[:])

                qrow = pp.tile([1, DG], F32, tag="q", bufs=1)
                for c2 in range(4):
                    pa = pp.tile([128, DG], F32, tag="big", bufs=3)
                    nc.tensor.matmul(pa[:], wa[:, c2 * 128:(c2 + 1) * 128],
                                     actT[:, gs0:gs0 + DG], start=True, stop=True)
                    t2 = tlp.tile([128, DG], F32, tag="tl")
                    nc.vector.tensor_tensor(t2[:], Yb[c2][:], rstd2b[:], OP.mult)
                    t3 = tlp.tile([128, DG], F32, tag="tl")
                    nc.vector.scalar_tensor_tensor(t3[:], t2[:], g2c[:, c2:c2 + 1],
                                                   pa[:], OP.mult, OP.add)
                    sa = sap.tile([128, DG], BF16, tag="sa")
                    nc.scalar.activation(sa[:], t3[:], AF.Relu)
                    nc.tensor.matmul(qrow[:], wqc[:, c2:c2 + 1], sa[:],
                                     start=(c2 == 0), stop=(c2 == 3))

                nc.scalar.activation(qout[0:1, gs0:gs0 + DG], qrow[:],
                                     AF.Copy, bias=bq)

            # software pipeline: C(g-1) overlaps A(g+1)/B(g) on other engines
            state = {}
            for g in range(GROUPS):
                zt = stage_A(g)
                if g >= 1:
                    stage_C(g - 1, *state[g - 1])
                state[g] = stage_B(g, zt)
            stage_C(GROUPS - 1, *state[GROUPS - 1])

            out_ap = OUT.rearrange("(a b) o -> b (a o)", b=1)
            nc.sync.dma_start(out_ap, qout[:])
    nc.compile()
    return nc


def kernel(**inputs):
    weights, percore, perms, meta = _preprocess(**inputs)

    key = (meta["NSRC_PAD"], meta["K"], meta["tot_k"])
    if key not in _KERNEL_CACHE:
        _KERNEL_CACHE[key] = _build(meta)
    nc = _KERNEL_CACHE[key]

    in_maps = []
    for c in range(N_CORES):
        m = dict(weights)
        m["xsrc"] = percore["xsrc"][c]
        m["idx"] = percore["idx"][c]
        m["dinvd"] = percore["dinvd"][c]
        m["actT"] = percore["actT"][c]
        in_maps.append(m)

    trace = os.environ.get("KERNEL_TRACE", "0") == "1"
    kw = {}
    if trace:
        import types, contextlib, ctypes
        if "antenv.axon_hooks" not in sys.modules:
            lib = ctypes.CDLL("/opt/axon/libaxon_pjrt.so")
            lib.axon_start_nrt_profile.argtypes = [
                ctypes.POINTER(ctypes.c_int64), ctypes.c_size_t]
            lib.axon_start_nrt_profile.restype = ctypes.c_int64
            lib.axon_stop_nrt_profile.argtypes = [ctypes.c_char_p]
            lib.axon_stop_nrt_profile.restype = ctypes.c_int64

            @contextlib.contextmanager
            def _hook(output_dir, device_ids):
                import jax
                jax.devices()
                if device_ids:
                    ids = (ctypes.c_int64 * len(device_ids))(*device_ids)
                    rc = lib.axon_start_nrt_profile(ids, len(device_ids))
                else:
                    rc = lib.axon_start_nrt_profile(None, 0)
                if rc != 0:
                    raise RuntimeError(f"axon_start_nrt_profile rc={rc}")
                try:
                    yield
                finally:
                    n = lib.axon_stop_nrt_profile(str(output_dir).encode())
                    print(f"profile: {n} file(s) written to {output_dir}",
                          file=sys.stderr)

            mod = types.ModuleType("antenv.axon_hooks")
            mod.get_axon_ntff_profile_hook = lambda: _hook
            sys.modules["antenv.axon_hooks"] = mod
        kw = dict(trace=True,
                  tmpdir=os.environ.get("KERNEL_TRACE_DIR") or None)

    res = run_bass_kernel_spmd(nc, in_maps, list(range(N_CORES)), **kw)
    if trace and res.exec_time_ns is not None:
        print(f"HW exec time: {res.exec_time_ns} ns")

    out = np.empty((N_AGENTS, 1), np.float32)
    for c in range(N_CORES):
        q = res.results[c]["q"]          # [PA, 1], indegree-sorted order
        blk = out[c * PA:(c + 1) * PA]
        blk[perms[c]] = q
    return out


# revision 14
# speedup vs baseline: 1.1143x; 1.1068x over previous
"""Trainium2 Bass kernel for nn_CriticNetwork (GCN message passing + critic MLP).

Strategy (8 NeuronCores, SPMD, no collectives):
  - Only agg[agent_idx] rows are consumed downstream, so message passing is
    pruned to edges whose destination is an agent node (dead-code elimination).
  - GCN transform is algebraically moved after aggregation:
    A_hat @ (x W) == (A_hat @ x) W, so we aggregate 128-dim x rows.
  - Agents are sharded evenly: core c owns agents [c*2048, (c+1)*2048), sorted
    by indegree so fixed-K gather tiles are tight. Each core gathers the
    (dinv[src]-prescaled, bf16) x rows of its agents' in-edges with dma_gather
    (chunked across the 4 SWDGE queues to avoid descriptor-ring stalls),
    accumulates them with identity-matmuls into PSUM (giving agg^T
    feature-major), applies dinv[dst], then runs the critic head entirely
    feature-major with bf16 matmuls. LayerNorm mean-centering is folded into
    W1/W2 host-side (exact); the +b2c term is injected into the L2 PSUM via a
    rank-1 matmul with the std1 row so both LN variances reduce to a single
    ones-matrix matmul over squared activations.
  - Host does only index bookkeeping + weight folding; all O(E*dim) and
    O(A*dim^2) math runs on device.
"""
import os
import sys

sys.path.insert(0, "/opt/trn_rl_repo")

import numpy as np
import ml_dtypes

import concourse.bass as bass
import concourse.tile as tile
import concourse.mybir as mybir
from concourse import bacc
from concourse.bass_utils import run_bass_kernel_spmd

# ---- problem constants (hardcoded per spec) ----
N_NODES = 50000
DIM = 128          # IN_DIM
HID = 256
F1 = 1024
F2 = 512
NACT = 64
N_EDGES = 800000
N_AGENTS = 16384
N_CORES = 8
PA = N_AGENTS // N_CORES      # 2048 agents per core
TILES = PA // 128             # 16 d-tiles per core
GROUPS = 4                    # head processed in 4 groups of 512 agents
DG = PA // GROUPS             # 512
EPS = 1e-5
CHUNK_K = 12                  # max k-blocks (128 rows each) per dma_gather

F32 = mybir.dt.float32
F32R = mybir.dt.float32r
BF16 = mybir.dt.bfloat16
I16 = mybir.dt.int16
AF = mybir.ActivationFunctionType
OP = mybir.AluOpType

_KERNEL_CACHE = {}


def _wrap_idxs(idx_flat):
    """dma_gather index layout: index j lives at [j%16, j//16], replicated
    across the 8 groups of 16 partitions. idx_flat length must be %16==0."""
    arr = idx_flat.reshape(-1, 16).T.astype(np.int16)   # [16, n/16]
    return np.tile(arr, (8, 1))                          # [128, n/16]


def _preprocess(x, action, W_gcn, b_gcn, W1, b1, g1, beta1, W2, b2, g2, beta2,
                Wa, ba, Wq, bq, edge_index, agent_idx):
    f32 = np.float32
    x = np.asarray(x, f32); action = np.asarray(action, f32)
    edge_index = np.asarray(edge_index); agent_idx = np.asarray(agent_idx)
    W_gcn = np.asarray(W_gcn, f32); b_gcn = np.asarray(b_gcn, f32)
    W1 = np.asarray(W1, f32); b1 = np.asarray(b1, f32)
    g1 = np.asarray(g1, f32); beta1 = np.asarray(beta1, f32)
    W2 = np.asarray(W2, f32); b2 = np.asarray(b2, f32)
    g2 = np.asarray(g2, f32); beta2 = np.asarray(beta2, f32)
    Wa = np.asarray(Wa, f32); ba = np.asarray(ba, f32)
    Wq = np.asarray(Wq, f32); bq = np.asarray(bq, f32)

    assert np.all(beta1 == 0.0) and np.all(g1 > 0.0), \
        "kernel fast path requires beta1==0 and g1>0 (module init guarantees this)"

    N = N_NODES
    loops = np.arange(N, dtype=edge_index.dtype)
    src_all = np.concatenate([edge_index[0], loops])
    dst_all = np.concatenate([edge_index[1], loops])
    deg = np.bincount(dst_all, minlength=N).astype(np.int64)
    dinv = (1.0 / np.sqrt(np.maximum(deg, 1.0))).astype(f32)

    order = np.argsort(dst_all, kind="stable")
    src_sorted = src_all[order]
    starts = np.searchsorted(dst_all[order], np.arange(N + 1))

    # per-core agent partition + indegree sort
    perms, agents_p, indegs = [], [], []
    for c in range(N_CORES):
        ag = agent_idx[c * PA:(c + 1) * PA]
        ind = deg[ag]
        perm = np.argsort(ind, kind="stable")
        perms.append(perm)
        agents_p.append(ag[perm])
        indegs.append(ind[perm])

    # shared per-tile K (max over cores so the SPMD program is identical)
    K = np.zeros(TILES, np.int64)
    for c in range(N_CORES):
        K = np.maximum(K, indegs[c].reshape(TILES, 128).max(axis=1))
    K = np.maximum(K, 1).astype(int)
    koff = np.concatenate([[0], np.cumsum(K)])
    tot_k = int(koff[-1])

    # per-core edge tables (src node per slot; -1 = pad)
    slot_src = []   # [TILES] of [K[t], 128] global node ids (-1 pad)
    for c in range(N_CORES):
        ag = agents_p[c]; ind = indegs[c]
        per_tile = []
        for t in range(TILES):
            kt = K[t]
            tbl = np.full((kt, 128), -1, np.int64)
            for p in range(128):
                a = int(ag[t * 128 + p]); d = int(ind[t * 128 + p])
                s = starts[a]
                tbl[:d, p] = src_sorted[s:s + d]
            per_tile.append(tbl)
        slot_src.append(per_tile)

    # per-core compaction of source rows
    uniqs, n_us = [], []
    for c in range(N_CORES):
        allsrc = np.concatenate([t.ravel() for t in slot_src[c]])
        uniq = np.unique(allsrc[allsrc >= 0])
        uniqs.append(uniq); n_us.append(len(uniq))
    NSRC_PAD = max(n_us) + 1
    assert NSRC_PAD <= 32767, f"NSRC_PAD={NSRC_PAD} exceeds int16 index range"

    xsrc_list, idx_list, dinvd_list, actT_list = [], [], [], []
    for c in range(N_CORES):
        uniq = uniqs[c]; n_u = n_us[c]
        xs = np.zeros((NSRC_PAD, DIM), f32)
        xs[:n_u] = x[uniq] * dinv[uniq][:, None]
        xsrc_list.append(xs.astype(ml_dtypes.bfloat16))
        zero_idx = n_u
        # remap global src -> local compact index
        wrapped = []
        for t in range(TILES):
            tbl = slot_src[c][t]
            loc = np.searchsorted(uniq, np.maximum(tbl, 0))
            loc = np.where(tbl >= 0, loc, zero_idx).astype(np.int64)
            wrapped.append(_wrap_idxs(loc.ravel()))   # [128, K[t]*8]
        idx_list.append(np.concatenate(wrapped, axis=1).astype(np.int16))
        # per-tile diagonal of dinv[dst]: folded into the aggregation matmul
        dg = np.zeros((128, TILES * 128), np.float32)
        dvals = dinv[agents_p[c]]
        for t in range(TILES):
            np.fill_diagonal(dg[:, t * 128:(t + 1) * 128],
                             dvals[t * 128:(t + 1) * 128])
        dinvd_list.append(dg.astype(ml_dtypes.bfloat16))
        # augmented action^T: row 64 = ones (for the fused +ba+beta2 bias)
        at = np.ones((NACT + 1, PA), f32)
        at[:NACT] = action[c * PA:(c + 1) * PA][perms[c]].T
        actT_list.append(at.astype(ml_dtypes.bfloat16))

    # ---- weight folding (exact algebra) ----
    w1m = W1.mean(axis=1)                       # [HID]
    W1f = W1 - w1m[:, None]                     # zero col-mean
    b1c = b1 - b1.mean()
    W2g = g1[:, None] * W2
    w2gm = W2g.mean(axis=1)
    W2f = W2g - w2gm[:, None]
    b2c = b2 - b2.mean()
    bb = ba + beta2

    def ktile_pack(W, kt, fdim):   # [kt*128, fdim] -> [128, kt*fdim]
        return np.ascontiguousarray(
            W.reshape(kt, 128, fdim).transpose(1, 0, 2).reshape(128, kt * fdim))

    bf16 = ml_dtypes.bfloat16
    wa_aug = np.empty((NACT + 1, F2), f32)
    wa_aug[:NACT] = Wa
    wa_aug[NACT] = bb
    wqc = np.ascontiguousarray(Wq.reshape(4, 128).T)          # [128, 4]

    weights = {
        "wgcn": W_gcn.astype(bf16),                           # [128, 256]
        "w1": ktile_pack(W1f, 2, F1).astype(bf16),            # [128, 2048]
        "w2": ktile_pack(W2f, 8, F2).astype(bf16),            # [128, 4096]
        "wa": wa_aug.astype(bf16),                            # [65, 512]
        "wqc": wqc.astype(bf16),                              # [128, 4]
        "bgcn_col": np.ascontiguousarray(b_gcn.reshape(2, 128).T),
        "b1c32_col": np.ascontiguousarray((b1c / 32.0).reshape(8, 128).T),
        "b1_col": np.ascontiguousarray(b1c.reshape(8, 128).T),
        "g2_col": np.ascontiguousarray(g2.reshape(4, 128).T),
        "b2c_row": b2c.reshape(1, F2).astype(bf16),           # [1, 512]
        "onesmat_in": np.ones((128, 128), bf16),
        "onesf2_in": np.full((128, 128), 1.0 / F2, bf16),     # 1/512 exact
    }
    meta = dict(NSRC_PAD=NSRC_PAD, K=tuple(int(k) for k in K),
                koff=tuple(int(o) for o in koff), tot_k=tot_k,
                bq=float(bq[0]))
    percore = dict(xsrc=xsrc_list, idx=idx_list, dinvd=dinvd_list,
                   actT=actT_list)
    return weights, percore, perms, meta


def _build(meta):
    NSRC_PAD = meta["NSRC_PAD"]; K = meta["K"]; koff = meta["koff"]
    tot_k = meta["tot_k"]; bq = meta["bq"]
    KMAX = max(K)

    nc = bacc.Bacc("TRN2", target_bir_lowering=False, debug=False,
                   num_devices=N_CORES, num_swdge_queues=4,
                   dynamic_dma_scratch_size=32768)
    dram = {}
    def din(name, shape, dt):
        dram[name] = nc.dram_tensor(name, shape, dt, kind="ExternalInput").ap()
        return dram[name]

    KA = koff[8]      # idx columns for tiles 0-7 (loaded first via gpsimd)
    xsrc = din("xsrc", [NSRC_PAD, DIM], BF16)
    idxs = din("idx", [128, 8 * tot_k], I16)
    dinvd = din("dinvd", [128, TILES * 128], BF16)
    actT_d = din("actT", [NACT + 1, PA], BF16)
    wgcn_d = din("wgcn", [128, HID], BF16)
    w1_d = din("w1", [128, 2 * F1], BF16)
    w2_d = din("w2", [128, 8 * F2], BF16)
    wa_d = din("wa", [NACT + 1, F2], BF16)
    wqc_d = din("wqc", [128, 4], BF16)
    bgcn_d = din("bgcn_col", [128, 2], F32)
    b1c32_d = din("b1c32_col", [128, 8], F32)
    b1_d = din("b1_col", [128, 8], F32)
    g2_d = din("g2_col", [128, 4], F32)
    b2c_d = din("b2c_row", [1, F2], BF16)
    onesmat_d = din("onesmat_in", [128, 128], BF16)
    onesf2_d = din("onesf2_in", [128, 128], BF16)
    OUT = nc.dram_tensor("q", [PA, 1], F32, kind="ExternalOutput").ap()

    with tile.TileContext(nc) as tc:
        with tc.tile_pool(name="w", bufs=1) as wp, \
             tc.tile_pool(name="zp", bufs=3) as zp, \
             tc.tile_pool(name="s1p", bufs=10) as s1p, \
             tc.tile_pool(name="sqp", bufs=2) as sqp, \
             tc.tile_pool(name="yap", bufs=5) as yap, \
             tc.tile_pool(name="ya2p", bufs=2) as ya2p, \
             tc.tile_pool(name="tlp", bufs=3) as tlp, \
             tc.tile_pool(name="sap", bufs=5) as sap, \
             tc.tile_pool(name="vec", bufs=3) as vec, \
             tc.tile_pool(name="bcp", bufs=3) as bcp, \
             tc.tile_pool(name="ps", bufs=1, space="PSUM") as pp:

            # ---------- preload ----------
            # idx for tiles 0-7 via gpsimd's own SWDGE queue: no cross-engine
            # wait before the first gather.  Rest via sync/scalar HWDGE.
            idxA = wp.tile([128, 8 * KA], I16)
            nc.gpsimd.dma_start(idxA[:], idxs[:, :8 * KA])
            idxB = wp.tile([128, 8 * (tot_k - KA)], I16)
            nc.sync.dma_start(idxB[:], idxs[:, 8 * KA:])
            w2 = wp.tile([128, 8 * F2], BF16); nc.sync.dma_start(w2[:], w2_d[:])
            wa = wp.tile([NACT + 1, F2], BF16); nc.sync.dma_start(wa[:], wa_d[:])
            wqc = wp.tile([128, 4], BF16); nc.sync.dma_start(wqc[:], wqc_d[:])
            bgcn = wp.tile([128, 2], F32); nc.sync.dma_start(bgcn[:], bgcn_d[:])
            b1c32 = wp.tile([128, 8], F32); nc.sync.dma_start(b1c32[:], b1c32_d[:])
            b1c = wp.tile([128, 8], F32); nc.sync.dma_start(b1c[:], b1_d[:])
            g2c = wp.tile([128, 4], F32); nc.sync.dma_start(g2c[:], g2_d[:])
            b2cr = wp.tile([1, F2], BF16); nc.sync.dma_start(b2cr[:], b2c_d[:])

            diag = wp.tile([128, TILES * 128], BF16)
            nc.scalar.dma_start(diag[:], dinvd[:])
            w1 = wp.tile([128, 2 * F1], BF16); nc.scalar.dma_start(w1[:], w1_d[:])
            wgcn = wp.tile([128, HID], BF16); nc.scalar.dma_start(wgcn[:], wgcn_d[:])
            actT = wp.tile([NACT + 1, PA], BF16); nc.scalar.dma_start(actT[:], actT_d[:])
            onesm = wp.tile([128, 128], BF16); nc.scalar.dma_start(onesm[:], onesmat_d[:])
            onesf2 = wp.tile([128, 128], BF16); nc.scalar.dma_start(onesf2[:], onesf2_d[:])

            eps_t = wp.tile([128, 1], F32); nc.gpsimd.memset(eps_t[:], EPS)
            zeros = wp.tile([128, DG], F32); nc.gpsimd.memset(zeros[:], 0.0)
            aggb = wp.tile([128, PA], BF16)      # agg^T, feature-major
            qout = wp.tile([1, PA], F32)
            # dedicated edge buffer per tile: gathers never wait on reuse
            etile = [wp.tile([128, K[t] * 128], BF16, name=f"etile{t}")
                     for t in range(TILES)]

            qn = [0]   # SWDGE queue round-robin counter

            def idx_ap(col0, col1):
                if col1 <= KA:
                    return idxA[:, 8 * col0:8 * col1]
                return idxB[:, 8 * (col0 - KA):8 * (col1 - KA)]

            def emit_gather(t):
                kt = K[t]
                e = etile[t]
                nchunk = (kt + CHUNK_K - 1) // CHUNK_K
                bounds = [kt * i // nchunk for i in range(nchunk + 1)]
                for ci in range(nchunk):
                    c0, c1 = bounds[ci], bounds[ci + 1]
                    e3 = e[:, c0 * 128:c1 * 128].rearrange(
                        "p (k e) -> p k e", e=128)
                    nc.gpsimd.dma_gather(
                        e3, xsrc[:], idx_ap(koff[t] + c0, koff[t] + c1),
                        128 * (c1 - c0), 128 * (c1 - c0), DIM,
                        single_packet=False, queue_num=qn[0] % 4)
                    qn[0] += 1

            def stage_A(g):
                """aggregation (dinv[dst] folded into diag rhs) + z transform"""
                gs0 = g * DG
                for tl in range(4):
                    t = g * 4 + tl
                    kt = K[t]
                    e = etile[t]
                    aps = pp.tile([128, 128], F32, tag="agg", bufs=2)
                    for k in range(kt):
                        nc.tensor.matmul(aps[:], e[:, k * 128:(k + 1) * 128],
                                         diag[:, t * 128:(t + 1) * 128],
                                         start=(k == 0), stop=(k == kt - 1))
                    nc.scalar.activation(aggb[:, t * 128:(t + 1) * 128],
                                         aps[:], AF.Copy)
                zt = []
                for h in range(2):
                    zps = pp.tile([128, DG], F32, tag="big", bufs=3)
                    nc.tensor.matmul(zps[:], wgcn[:, h * 128:(h + 1) * 128],
                                     aggb[:, gs0:gs0 + DG], start=True, stop=True)
                    z = zp.tile([128, DG], BF16, tag="z")
                    nc.scalar.activation(z[:], zps[:], AF.Relu,
                                         bias=bgcn[:, h:h + 1])
                    zt.append(z)
                return zt

            def stage_B(g, zt):
                """L1 + LN1 stats (mean folded into W1f/b1c)
                var1 = sum over F1 of ((x1c + b1c)/32)^2  (1/32^2 = 1/F1)"""
                ps_sq1 = pp.tile([128, DG], F32, tag="stat", bufs=2)
                s1r = []
                for c in range(8):
                    lp = pp.tile([128, DG], F32, tag="big", bufs=3)
                    nc.tensor.matmul(lp[:], w1[:, c * 128:c * 128 + 128],
                                     zt[0][:], start=True, stop=False)
                    nc.tensor.matmul(lp[:], w1[:, F1 + c * 128:F1 + c * 128 + 128],
                                     zt[1][:], start=False, stop=True)
                    sq = sqp.tile([128, DG], BF16, tag="sq")
                    nc.scalar.activation(sq[:], lp[:], AF.Square,
                                         bias=b1c32[:, c:c + 1], scale=1.0 / 32.0)
                    nc.tensor.matmul(ps_sq1[:], onesm[:], sq[:],
                                     start=(c == 0), stop=(c == 7))
                    sr = s1p.tile([128, DG], BF16, tag="s1")
                    if c % 2 == 0:
                        nc.scalar.activation(sr[:], lp[:], AF.Relu,
                                             bias=b1c[:, c:c + 1])
                    else:
                        nc.vector.scalar_tensor_tensor(
                            sr[:], lp[:], b1c[:, c:c + 1], zeros[:],
                            OP.add, OP.max)
                    s1r.append(sr)

                std1 = vec.tile([128, DG], F32, tag="v")
                nc.scalar.activation(std1[:], ps_sq1[:], AF.Sqrt, bias=eps_t[:])
                rstd1b = bcp.tile([128, DG], F32, tag="bc")
                nc.vector.reciprocal_approx_fast(rstd1b[:], std1[:])
                std1row = vec.tile([1, DG], BF16, tag="vrow", bufs=3)
                nc.scalar.activation(std1row[:], std1[0:1, :], AF.Copy)
                return s1r, rstd1b, std1row

            def stage_C(g, s1r, rstd1b, std1row):
                """L2 (+rank-1 b2c*std1) + LN2 + tail + q
                yb = (W2f@s1r + b2c x std1) * rstd1 == x2c + b2c (zero-mean)
                var2 = (ones/F2) @ yb^2"""
                gs0 = g * DG
                ps_s2 = pp.tile([128, DG], F32, tag="stat", bufs=2)
                Yb = []
                for c2 in range(4):
                    lp = pp.tile([128, DG], F32, tag="big", bufs=3)
                    for kt8 in range(8):
                        nc.tensor.matmul(
                            lp[:], w2[:, kt8 * F2 + c2 * 128:kt8 * F2 + c2 * 128 + 128],
                            s1r[kt8][:], start=(kt8 == 0), stop=False)
                    nc.tensor.matmul(lp[:], b2cr[:, c2 * 128:(c2 + 1) * 128],
                                     std1row[:], start=False, stop=True)
                    yb = yap.tile([128, DG], BF16, tag="ya")
                    nc.vector.tensor_tensor(yb[:], lp[:], rstd1b[:], OP.mult)
                    y2 = ya2p.tile([128, DG], BF16, tag="ya2")
                    nc.vector.tensor_tensor(y2[:], yb[:], yb[:], OP.mult)
                    nc.tensor.matmul(ps_s2[:], onesf2[:], y2[:],
                                     start=(c2 == 0), stop=(c2 == 3))
                    Yb.append(yb)

                std2 = vec.tile([128, DG], F32, tag="v")
                nc.scalar.activation(std2[:], ps_s2[:], AF.Sqrt, bias=eps_t[:])
                rstd2b = bcp.tile([128, DG], F32, tag="bc")
                nc.vector.reciprocal_approx_fast(rstd2b[:], std2[:])

                qrow = pp.tile([1, DG], F32, tag="q", bufs=1)
                for c2 in range(4):
                    pa = pp.tile([128, DG], F32, tag="big", bufs=3)
                    nc.tensor.matmul(pa[:], wa[:, c2 * 128:(c2 + 1) * 128],
                                     actT[:, gs0:gs0 + DG], start=True, stop=True)
                    t2 = tlp.tile([128, DG], F32, tag="tl")
                    nc.vector.tensor_tensor(t2[:], Yb[c2][:], rstd2b[:], OP.mult)
                    t3 = tlp.tile([128, DG], F32, tag="tl")
                    nc.vector.scalar_tensor_tensor(t3[:], t2[:], g2c[:, c2:c2 + 1],
                                                   pa[:], OP.mult, OP.add)
                    sa = sap.tile([128, DG], BF16, tag="sa")
                    nc.scalar.activation(sa[:], t3[:], AF.Relu)
                    nc.tensor.matmul(qrow[:], wqc[:, c2:c2 + 1], sa[:],
                                     start=(c2 == 0), stop=(c2 == 3))

                nc.scalar.activation(qout[0:1, gs0:gs0 + DG], qrow[:],
                                     AF.Copy, bias=bq)

            # all gathers issued up-front on the gpsimd stream (dedicated
            # buffers: nothing blocks them); compute follows in plain order
            for t in range(TILES):
                emit_gather(t)
            for g in range(GROUPS):
                zt = stage_A(g)
                s1r, rstd1b, std1row = stage_B(g, zt)
                stage_C(g, s1r, rstd1b, std1row)

            out_ap = OUT.rearrange("(a b) o -> b (a o)", b=1)
            nc.sync.dma_start(out_ap, qout[:])
    nc.compile()
    return nc


def kernel(**inputs):
    weights, percore, perms, meta = _preprocess(**inputs)

    key = (meta["NSRC_PAD"], meta["K"], meta["tot_k"])
    if key not in _KERNEL_CACHE:
        _KERNEL_CACHE[key] = _build(meta)
    nc = _KERNEL_CACHE[key]

    in_maps = []
    for c in range(N_CORES):
        m = dict(weights)
        m["xsrc"] = percore["xsrc"][c]
        m["idx"] = percore["idx"][c]
        m["dinvd"] = percore["dinvd"][c]
        m["actT"] = percore["actT"][c]
        in_maps.append(m)

    trace = os.environ.get("KERNEL_TRACE", "0") == "1"
    kw = {}
    if trace:
        import types, contextlib, ctypes
        if "antenv.axon_hooks" not in sys.modules:
            lib = ctypes.CDLL("/opt/axon/libaxon_pjrt.so")
            lib.axon_start_nrt_profile.argtypes = [
                ctypes.POINTER(ctypes.c_int64), ctypes.c_size_t]
            lib.axon_start_nrt_profile.restype = ctypes.c_int64
            lib.axon_stop_nrt_profile.argtypes = [ctypes.c_char_p]
            lib.axon_stop_nrt_profile.restype = ctypes.c_int64

            @contextlib.contextmanager
            def _hook(output_dir, device_ids):
                import jax
                jax.devices()
                if device_ids:
                    ids = (ctypes.c_int64 * len(device_ids))(*device_ids)
                    rc = lib.axon_start_nrt_profile(ids, len(device_ids))
                else:
                    rc = lib.axon_start_nrt_profile(None, 0)
                if rc != 0:
                    raise RuntimeError(f"axon_start_nrt_profile rc={rc}")
                try:
                    yield
                finally:
                    n = lib.axon_stop_nrt_profile(str(output_dir).encode())
                    print(f"profile: {n} file(s) written to {output_dir}",
                          file=sys.stderr)

            mod = types.ModuleType("antenv.axon_hooks")
            mod.get_axon_ntff_profile_hook = lambda: _hook
            sys.modules["antenv.axon_hooks"] = mod
        kw = dict(trace=True,
                  tmpdir=os.environ.get("KERNEL_TRACE_DIR") or None)

    res = run_bass_kernel_spmd(nc, in_maps, list(range(N_CORES)), **kw)
    if trace and res.exec_time_ns is not None:
        print(f"HW exec time: {res.exec_time_ns} ns")

    out = np.empty((N_AGENTS, 1), np.float32)
    for c in range(N_CORES):
        q = res.results[c]["q"]          # [PA, 1], indegree-sorted order
        blk = out[c * PA:(c + 1) * PA]
        blk[perms[c]] = q
    return out
